# revision 1
# baseline (speedup 1.0000x reference)
"""Trainium2 Bass kernel for nn_Attention_37056977830181.

Head-sharded (tensor-parallel) multi-head attention over 8 NeuronCores:
each core computes 2 of the 16 heads end-to-end (QKV projection, per-head
RMSNorm, softmax attention, output-projection partial sum); the host sums
the 8 partial projection outputs.

Layout strategy (all big matmuls in bf16, fp32 accumulation):
  - x is pre-transposed on the host to xT [B, C, N] so the QKV matmuls
    contract over C on the partition dimension.
  - q, k are produced directly in transposed form qT/kT [dd=128, N]
    (lhsT = W chunk, rhs = xT chunk), so scoresT = kT.T-free matmul needs
    no on-chip transposes. RMSNorm runs in this layout: per-head partition
    sums via a ones-block matmul, rsqrt via ACT-sqrt + DVE-reciprocal,
    broadcast back via 0-step-free-dim DMA.
  - v is produced in natural layout [N, dd] (lhsT = xT chunk, rhs = W),
    with a ones-column appended so the attention matmul's 65th output row
    accumulates the softmax denominator.
  - scoresT [j, i] per head; softmax needs no max-subtraction (scores are
    bounded ~N(0,1) after RMSNorm); exp on ACT reads PSUM directly and
    emits bf16 p-tiles; normalization is folded in after attn@v.
"""

import dataclasses
import numpy as np
import ml_dtypes

B, N, C = 4, 2048, 1024
H = 16
D = C // H
SCALE = D**-0.5
EPS = 1e-6
N_CORES = 8
HPC = H // N_CORES  # heads per core = 2
DD = HPC * D  # per-core channel block = 128

bf16 = ml_dtypes.bfloat16

_COMPILED = {}


def _row_bcast(ap, rows):
    """View a [1, F] SBUF AP as [1, rows, F] with a 0-step middle dim, so a
    DMA with a [rows, F] destination replicates the row across partitions."""
    f = ap.shape[-1]
    (pstep, pcount), (estep, ecount) = ap.ap[0], ap.ap[-1]
    assert pcount == 1 and ecount == f
    return dataclasses.replace(ap, ap=[[pstep, 1], [0, rows], [estep, f]])


_WAIT_CAPS = {}
_WAIT_SKIP = {"EventSemaphore", "Call", "ISA", "UnconditionalBranch"}
_WAIT_DEFAULT_CAP = 1
_NOP_CAP = 1


def _split_waits(nc):
    """Walrus's per-instruction-struct sync-wait slots are limited (e.g. the
    self-loading-weights matmul struct takes 1, ACTIVATE takes 2).  Move
    excess waits onto no-op instructions inserted just before, on the same
    engine, preserving execution order semantics."""
    import concourse.mybir as mybir

    nid = [0]
    for f in nc.m.functions:
        for bb in f.blocks:
            out = []
            for inst in bb.instructions:
                si = inst.sync_info
                waits = list(si.on_wait) if si is not None and si.on_wait else []
                cap = (
                    10**9
                    if inst.opcode in _WAIT_SKIP
                    else _WAIT_CAPS.get(inst.opcode, _WAIT_DEFAULT_CAP)
                )
                if len(waits) > cap:
                    excess = waits[: len(waits) - cap]
                    keep = waits[len(waits) - cap :]
                    for j in range(0, len(excess), _NOP_CAP):
                        nop = mybir.InstNoOp(
                            name=f"I-waitsplit-{nid[0]}", ins=[], outs=[]
                        )
                        nid[0] += 1
                        nop.engine = inst.engine
                        nop.bass_nofuse = True
                        nop.sync_info = mybir.SyncInfo(
                            on_wait=excess[j : j + _NOP_CAP], on_update=[]
                        )
                        out.append(nop)
                    inst.sync_info = mybir.SyncInfo(
                        on_wait=keep, on_update=list(si.on_update or [])
                    )
                out.append(inst)
            bb.instructions[:] = out


def build_program(reps=1, hw_loop=0):
    """reps: python-unrolled repetitions.  hw_loop: if >0, wrap the body in a
    Tile For_i hardware loop of that many iterations (for timing runs)."""
    import contextlib
    import concourse.bass as bass
    import concourse.mybir as mybir
    import concourse.tile as tile

    F32 = mybir.dt.float32
    BF16 = mybir.dt.bfloat16
    AF = mybir.ActivationFunctionType

    nc = bass.Bass(
        "TRN2",
        target_bir_lowering=False,
        debug=False,
        enable_asserts=True,
        num_devices=N_CORES,
    )

    xt_d = nc.dram_tensor("xt", [B, C, N], BF16, kind="ExternalInput").ap()
    wq_d = nc.dram_tensor("wq", [128, 1024], BF16, kind="ExternalInput").ap()
    wk_d = nc.dram_tensor("wk", [128, 1024], BF16, kind="ExternalInput").ap()
    wv_d = nc.dram_tensor("wv", [128, 1024], BF16, kind="ExternalInput").ap()
    pw_d = nc.dram_tensor("pw", [DD, C], BF16, kind="ExternalInput").ap()
    qw_d = nc.dram_tensor("qw", [128, 1], F32, kind="ExternalInput").ap()
    kw_d = nc.dram_tensor("kw", [128, 1], F32, kind="ExternalInput").ap()
    ones2_d = nc.dram_tensor("ones2", [128, 2], BF16, kind="ExternalInput").ap()
    onesrep_d = nc.dram_tensor("onesrep", [128, 128], BF16, kind="ExternalInput").ap()
    y_d = nc.dram_tensor("y", [B, N, C], F32, kind="ExternalOutput").ap()

    NKC = C // 128  # 8 contraction chunks
    NJC = N // 128  # 16 key chunks
    NIH = 4  # i-quarters of 512
    IW = N // NIH  # 512

    with tile.TileContext(nc) as tc:
        with (
            tc.tile_pool(name="const", bufs=1) as cpool,
            tc.tile_pool(name="xt", bufs=2) as xpool,
            tc.tile_pool(name="qk", bufs=2) as qkpool,
            tc.tile_pool(name="v", bufs=2) as vpool,
            tc.tile_pool(name="work", bufs=2) as wpool,
            tc.tile_pool(name="y", bufs=3) as ypool,
            tc.tile_pool(name="ps", bufs=1, space="PSUM") as ps,
        ):
            # --- constants ---
            w_sb = {}
            for name, dram in (("wq", wq_d), ("wk", wk_d), ("wv", wv_d)):
                t = cpool.tile([128, 1024], BF16, tag=f"c_{name}")
                nc.sync.dma_start(out=t[:], in_=dram)
                w_sb[name] = t
            pw_sb = cpool.tile([DD, C], BF16, tag="c_pw")
            nc.sync.dma_start(out=pw_sb[:], in_=pw_d)
            qw_sb = cpool.tile([128, 1], F32, tag="c_qw")
            nc.sync.dma_start(out=qw_sb[:], in_=qw_d)
            kw_sb = cpool.tile([128, 1], F32, tag="c_kw")
            nc.sync.dma_start(out=kw_sb[:], in_=kw_d)
            ones2_sb = cpool.tile([128, 2], BF16, tag="c_ones2")
            nc.sync.dma_start(out=ones2_sb[:], in_=ones2_d)
            onesrep_sb = cpool.tile([128, 128], BF16, tag="c_onesrep")
            nc.sync.dma_start(out=onesrep_sb[:], in_=onesrep_d)
            eps_sb = cpool.tile([128, 1], F32, tag="c_eps")
            nc.vector.memset(eps_sb[:], EPS)

            loop_ctx = tc.For_i(0, hw_loop, 1) if hw_loop else contextlib.nullcontext()
            with loop_ctx:
              for rep in range(reps):
                for b in range(B):
                    # --- load xT for this batch: 8 chunks [128, N] ---
                    xt_sb = []
                    for kc in range(NKC):
                        t = xpool.tile([128, N], BF16, tag=f"xt{kc}")
                        nc.sync.dma_start(
                            out=t[:], in_=xt_d[b, kc * 128 : (kc + 1) * 128, :]
                        )
                        xt_sb.append(t)

                    # --- Phase Q: qT / kT with RMSNorm, in T layout ---
                    # Raw qT/kT are staged to SBUF; per-head variances are
                    # computed via a block-diagonal ones matmul that emits the
                    # result already broadcast across each head's 64
                    # partitions; ALL rsqrt work happens in ONE ACT Sqrt (so
                    # the ACT table set switches only twice per batch instead
                    # of per-chunk) followed by one DVE reciprocal.
                    qnT = qkpool.tile([128, N], BF16, tag="qnT")
                    knT = qkpool.tile([128, N], BF16, tag="knT")
                    qraw = qkpool.tile([128, N], F32, tag="qraw", bufs=1)
                    kraw = qkpool.tile([128, N], F32, tag="kraw", bufs=1)
                    varb = qkpool.tile([128, 2 * N], F32, tag="varb", bufs=1)
                    for ti, (tname, wkey, rawT) in enumerate(
                        (("q", "wq", qraw), ("k", "wk", kraw))
                    ):
                        for ncq in range(N // 512):
                            sl = slice(ncq * 512, (ncq + 1) * 512)
                            vsl = slice(ti * N + ncq * 512, ti * N + (ncq + 1) * 512)
                            pq = ps.tile(
                                [128, 512], F32, tag="small0", name="pq"
                            )
                            for kc in range(NKC):
                                nc.tensor.matmul(
                                    pq[:],
                                    w_sb[wkey][:, kc * 128 : (kc + 1) * 128],
                                    xt_sb[kc][:, sl],
                                    start=(kc == 0),
                                    stop=(kc == NKC - 1),
                                )
                            nc.vector.tensor_copy(rawT[:, sl], pq[:])
                            sq = wpool.tile([128, 512], BF16, tag="sq", bufs=4)
                            nc.vector.tensor_mul(sq[:], pq[:], rawT[:, sl])
                            psums = ps.tile(
                                [128, 512], F32, tag="small0", name="psums"
                            )
                            nc.tensor.matmul(
                                psums[:], onesrep_sb[:], sq[:], start=True, stop=True
                            )
                            # drain on DVE: keep Q-phase work off the busy
                            # ACT queue (ACT ops here would sit behind the
                            # previous batch's softmax exps in FIFO order)
                            nc.vector.tensor_copy(varb[:, vsl], psums[:])
                    # one batched rsqrt per batch: 2 ACT table switches
                    # per batch instead of per-chunk thrash
                    nc.scalar.activation(
                        varb[:], varb[:], AF.Sqrt, bias=eps_sb[:], scale=1.0 / D
                    )
                    nc.vector.reciprocal(varb[:], varb[:])
                    for ti, (rawT, wcol, dstT) in enumerate(
                        ((qraw, qw_sb, qnT), (kraw, kw_sb, knT))
                    ):
                        for ncq in range(N // 512):
                            sl = slice(ncq * 512, (ncq + 1) * 512)
                            vsl = slice(ti * N + ncq * 512, ti * N + (ncq + 1) * 512)
                            import concourse.mybir as _mb
                            nc.vector.scalar_tensor_tensor(
                                dstT[:, sl],
                                rawT[:, sl],
                                wcol[:],
                                varb[:, vsl],
                                op0=_mb.AluOpType.mult,
                                op1=_mb.AluOpType.mult,
                            )

                    # --- Phase V: v in natural layout + ones column ---
                    v_tiles = []
                    for jc in range(NJC):
                        pvt = ps.tile([128, 512], F32, tag="small0", name="pv")
                        pv = pvt[:, 0:128]
                        for kc in range(NKC):
                            nc.tensor.matmul(
                                pv,
                                xt_sb[kc][:, jc * 128 : (jc + 1) * 128],
                                w_sb["wv"][:, kc * 128 : (kc + 1) * 128],
                                start=(kc == 0),
                                stop=(kc == NKC - 1),
                            )
                        vt = vpool.tile([128, 130], BF16, tag=f"v{jc}")
                        nc.vector.tensor_copy(vt[:, 0:64], pvt[:, 0:64])
                        nc.vector.tensor_copy(vt[:, 65:129], pvt[:, 64:128])
                        nc.vector.memset(vt[:, 64:65], 1.0)
                        nc.vector.memset(vt[:, 129:130], 1.0)
                        v_tiles.append(vt)

                    # --- Phase A + D: attention per i-quarter (IW=512) ---
                    for ih in range(NIH):
                        isl = slice(ih * IW, (ih + 1) * IW)
                        acc_h = []
                        for h in range(HPC):
                            acc_t = ps.tile([65, IW], F32, tag=f"acc{h}", name=f"acc_t{h}")
                            acc_h.append(acc_t)
                        # software-pipelined: attn@v runs one jc behind the
                        # scores+exp pair so PE never queues behind ACT
                        p_prev = None
                        for jc in range(NJC):
                            # both heads' scoresT side by side in one 2-bank
                            # tile so the exp runs 1024 wide
                            scs = ps.tile(
                                [128, 2 * IW], F32, tag="scs", bufs=2, name="scs"
                            )
                            for h in range(HPC):
                                hs = slice(h * 64, (h + 1) * 64)
                                nc.tensor.matmul(
                                    scs[:, h * IW : (h + 1) * IW],
                                    knT[hs, jc * 128 : (jc + 1) * 128],
                                    qnT[hs, isl],
                                    start=True,
                                    stop=True,
                                    tile_position=(h * 64, 0),
                                )
                            if p_prev is not None:
                                vt = v_tiles[jc - 1]
                                for h in range(HPC):
                                    nc.tensor.matmul(
                                        acc_h[h][:],
                                        vt[:, h * 65 : h * 65 + 65],
                                        p_prev[:, h * IW : (h + 1) * IW],
                                        start=(jc == 1),
                                        stop=False,
                                    )
                            p = wpool.tile([128, 2 * IW], BF16, tag="p", bufs=3)
                            nc.scalar.activation(p[:], scs[:], AF.Exp, scale=SCALE)
                            p_prev = p
                        vt = v_tiles[NJC - 1]
                        for h in range(HPC):
                            nc.tensor.matmul(
                                acc_h[h][:],
                                vt[:, h * 65 : h * 65 + 65],
                                p_prev[:, h * IW : (h + 1) * IW],
                                start=False,
                                stop=True,
                            )
                        # drain: normalize + project + store
                        rb2 = wpool.tile([128, IW], F32, tag="rb2", bufs=2)
                        for h in range(HPC):
                            rec = wpool.tile([1, IW], F32, tag=f"rec{h}", bufs=3)
                            nc.vector.reciprocal(rec[:], acc_h[h][64:65, :])
                            nc.sync.dma_start(
                                out=rb2[h * 64 : (h + 1) * 64, :],
                                in_=_row_bcast(rec[0:1, :], 64),
                            )
                        outTn = wpool.tile([128, IW], BF16, tag="outTn", bufs=3)
                        for h in range(HPC):
                            nc.vector.tensor_mul(
                                outTn[h * 64 : (h + 1) * 64, :],
                                acc_h[h][0:64, :],
                                rb2[h * 64 : (h + 1) * 64, :],
                            )
                        for mc in range(IW // 128):
                            for oc in range(C // 512):
                                yp = ps.tile(
                                    [128, 512], F32, tag="small1", name="yp"
                                )
                                nc.tensor.matmul(
                                    yp[:],
                                    outTn[:, mc * 128 : (mc + 1) * 128],
                                    pw_sb[:, oc * 512 : (oc + 1) * 512],
                                    start=True,
                                    stop=True,
                                )
                                ysb = ypool.tile([128, 512], F32, tag="ysb")
                                nc.vector.tensor_copy(ysb[:], yp[:])
                                qi0 = ih * IW + mc * 128
                                nc.sync.dma_start(
                                    out=y_d[b, qi0 : qi0 + 128, oc * 512 : (oc + 1) * 512],
                                    in_=ysb[:],
                                )
    _split_waits(nc)
    return nc


def _prepare_inputs(x, qkv_w, q_norm_w, k_norm_w, proj_w):
    """Host-side sharding/layout prep. Returns per-core input maps."""
    xt = np.ascontiguousarray(x.transpose(0, 2, 1)).astype(bf16)  # [B, C, N]
    qw_col = np.tile(q_norm_w, HPC).reshape(128, 1).astype(np.float32)
    kw_col = np.tile(k_norm_w, HPC).reshape(128, 1).astype(np.float32)
    ones2 = np.zeros((128, 2), bf16)
    ones2[0:64, 0] = 1
    ones2[64:128, 1] = 1
    onesrep = np.zeros((128, 128), bf16)
    onesrep[0:64, 0:64] = 1
    onesrep[64:128, 64:128] = 1

    in_maps = []
    for c in range(N_CORES):
        rows = slice(DD * c, DD * (c + 1))

        def pack(w):  # [128 rows, C] -> packed lhsT chunks [128, 1024]
            chunks = [
                np.ascontiguousarray(w[:, kc * 128 : (kc + 1) * 128].T)
                for kc in range(C // 128)
            ]
            return np.concatenate(chunks, axis=1).astype(bf16)

        in_maps.append(
            {
                "xt": xt,
                "wq": pack(qkv_w[0 * C :][rows, :]),
                "wk": pack(qkv_w[1 * C + DD * c : 1 * C + DD * (c + 1), :]),
                "wv": pack(qkv_w[2 * C + DD * c : 2 * C + DD * (c + 1), :]),
                "pw": np.ascontiguousarray(proj_w[:, rows].T).astype(bf16),
                "qw": qw_col,
                "kw": kw_col,
                "ones2": ones2,
                "onesrep": onesrep,
            }
        )
    return in_maps


def run_on_device(in_maps, reps=1, hw_loop=0):
    from concourse.bass_utils import run_bass_kernel_spmd

    key = (reps, hw_loop)
    if key not in _COMPILED:
        _COMPILED[key] = build_program(reps, hw_loop=hw_loop)
    nc = _COMPILED[key]
    res = run_bass_kernel_spmd(nc, in_maps, list(range(N_CORES)))
    return res


def kernel(x, qkv_w, q_norm_w, k_norm_w, proj_w, proj_b):
    x = np.asarray(x, np.float32)
    qkv_w = np.asarray(qkv_w, np.float32)
    proj_w = np.asarray(proj_w, np.float32)
    in_maps = _prepare_inputs(
        x, qkv_w, np.asarray(q_norm_w, np.float32), np.asarray(k_norm_w, np.float32), proj_w
    )
    res = run_on_device(in_maps, reps=1)
    y = np.zeros((B, N, C), np.float32)
    for c in range(N_CORES):
        y += res.results[c]["y"]
    y += np.asarray(proj_b, np.float32)[None, None, :]
    return y



# revision 21
# speedup vs baseline: 1.7170x; 1.7170x over previous
"""Trainium2 Bass kernel for nn_Attention_37056977830181.

Head-sharded (tensor-parallel) multi-head attention over 8 NeuronCores:
each core computes 2 of the 16 heads end-to-end (QKV projection, per-head
RMSNorm, softmax attention, output-projection partial sum); the host sums
the 8 partial projection outputs (stored bf16).

Layout strategy (all big matmuls in bf16, fp32 accumulation):
  - x is pre-transposed on the host to xT [B, C, N] so the QKV matmuls
    contract over C on the partition dimension.
  - q, k are produced directly in transposed form qT/kT [dd=128, N].
    RMSNorm runs in this layout: per-head partition sums via a ones-block
    matmul; rsqrt is computed as exp(-0.5*ln(var)) so the WHOLE kernel
    uses a single ACT table set (natural_log_exp) - no table switches.
    The softmax log2-domain prescale lambda = SCALE*log2(e) is folded into
    the q normalization, so attention scores come out in log2 domain.
  - v is produced in natural layout [N, dd] with a ones-column appended so
    the attention matmul's 65th output row accumulates the softmax
    denominator.
  - scoresT [j, i] per head; softmax needs no max-subtraction (scores are
    bounded after RMSNorm).  exp = 2^t is computed EITHER on ACT (Exp with
    scale=ln2, bias=-ln|k|) or offloaded to DVE for a subset of key-chunks:
    a tensor_scalar int32 conversion builds the Schraudolph seed
    y0 = 2^n*(1+f), then one custom 8-stage DVE op applies a factored-cubic
    mantissa correction (v+A)((v+B)^2+C) ~= 2^f/((1+f)|k|).  Both paths
    carry the same global factor 1/|k|, which cancels in the softmax
    normalization.
  - Emission is software-pipelined across batches: the ACT/DVE-heavy
    attention phase of batch b is interleaved with the PE-heavy QKV/V
    phase of batch b+1.
"""

import dataclasses
import numpy as np
import ml_dtypes

B, N, C = 4, 2048, 1024
H = 16
D = C // H
SCALE = D**-0.5
EPS = 1e-6
N_CORES = 8
HPC = H // N_CORES  # heads per core = 2
DD = HPC * D  # per-core channel block = 128

LOG2E = 1.4426950408889634
LN2 = 0.6931471805599453
LAMBDA = SCALE * LOG2E  # folds the 1/sqrt(D) scale + log2 domain into q

# factored-cubic correction constants: h(v)=2^(v-1)/v on [1,2],
# minimax cubic = k*(v+A)*((v-1)^2+C), k<0 (center pinned at 1 so the
# DVE op needs no src1 operand); DVE computes (AA-v)*((v-1)^2+C)*y0
# = 2^t/|k|; ACT matches with bias=-ln|k|.
EXP_K = -0.11368848320361466
EXP_A = -4.506197246547185  # AA = -A
EXP_C = 2.506638426614292
ACT_EXP_BIAS = 2.1742931744411185  # -ln|k|
MANT_MASK = float(np.frombuffer(np.uint32(0x007FFFFF).tobytes(), np.float32)[0])

# which key-chunks (per quarter index) compute exp on DVE instead of ACT
import os as _os
_KDVE = int(_os.environ.get("KDVE", "0"))  # DVE exp tiles per batch
_SPOTS = (8, 4, 12, 6, 10, 2, 14, 3)
DVE_JCS = {}
for _q in range(4):
    _n = _KDVE // 4 + (1 if _q < _KDVE % 4 else 0)
    DVE_JCS[_q] = tuple(sorted(_SPOTS[:_n]))

bf16 = ml_dtypes.bfloat16

_COMPILED = {}
_EXP2_OP = None


def _get_exp2_op():
    """Build (once) and register the custom DVE op with concourse.dve_ops."""
    global _EXP2_OP
    if _EXP2_OP is not None:
        return _EXP2_OP
    import concourse.dve_ops as dve_ops
    from concourse.dve_spec import Spec, Bin, Src0, Src1, C0, C1, C2, One, lower, _has_src1
    from concourse.dve_uop import AluOp, DveOpSpec

    w = Bin(AluOp.BITWISE_AND, Src0, C0)   # mantissa bits (C0 = denormal mask)
    v = Bin(AluOp.BITWISE_OR, w, One)      # value (1+f) in [1,2)
    q1 = Bin(AluOp.SUBTRACT, v, One)       # f = v - 1
    q2 = Bin(AluOp.MULTIPLY, q1, q1)
    q4 = Bin(AluOp.SUBTRACT, C1, v)        # AA - v  (C1 = s1)
    q3 = q2 + C2                           # + C     (C2 = imm2)
    q5 = Bin(AluOp.MULTIPLY, q4, q3)
    r = Bin(AluOp.MULTIPLY, q5, Src0)      # * y0

    def ref(in0, in1, s0, s1, imm2):
        bits = np.ascontiguousarray(in0).view(np.int32)
        vv = ((bits & 0x007FFFFF) | 0x3F800000).view(np.float32)
        a1 = (vv - np.float32(1.0)).astype(np.float32)
        a2 = (a1 * a1).astype(np.float32)
        a4 = (np.float32(s1) - vv).astype(np.float32)
        a3 = (a2 + np.float32(imm2)).astype(np.float32)
        a5 = (a4 * a3).astype(np.float32)
        return (a5 * in0).astype(np.float32)

    spec = Spec(body=r, reference=ref)
    shas = {}
    for ver in ("v3", "v4"):
        try:
            uops = lower(spec, ver=ver)
            shas[ver] = DveOpSpec(
                name="EXP2_FIXUP_ANT",
                opcode=dve_ops._CUSTOM_DVE_ROW_BASE + len(dve_ops.OPS),
                uops=uops,
                rd1_en=_has_src1(spec),
            ).sha(ver)
        except Exception:
            pass
    op = dve_ops.DveOp("EXP2_FIXUP_ANT", spec, subdim=False, uops_sha=shas)
    dve_ops._SUB_OPCODE_FOR_NAME[op.name] = (
        dve_ops._CUSTOM_DVE_ROW_BASE + len(dve_ops.OPS)
    )
    dve_ops.OPS.append(op)
    dve_ops.CUSTOM_DVE_SPECS[op.name] = op.spec
    _EXP2_OP = op
    return op


def _row_bcast(ap, rows):
    """View a [1, F] SBUF AP as [1, rows, F] with a 0-step middle dim, so a
    DMA with a [rows, F] destination replicates the row across partitions."""
    f = ap.shape[-1]
    (pstep, pcount), (estep, ecount) = ap.ap[0], ap.ap[-1]
    assert pcount == 1 and ecount == f
    return dataclasses.replace(ap, ap=[[pstep, 1], [0, rows], [estep, f]])


def _strided2(ap, ncols, stride, start=0):
    """View tile AP as [P, 2, ncols] selecting cols [start:start+ncols] and
    [start+stride : start+stride+ncols]."""
    (pstep, pcount), (estep, ecount) = ap.ap[0], ap.ap[-1]
    return dataclasses.replace(
        ap,
        offset=ap.offset + start * estep,
        ap=[[pstep, pcount], [estep * stride, 2], [estep, ncols]],
    )


_WAIT_CAPS = {}
_WAIT_SKIP = {"EventSemaphore", "Call", "ISA", "UnconditionalBranch"}
_WAIT_DEFAULT_CAP = 1
_NOP_CAP = 1


def _split_waits(nc):
    """Walrus's per-instruction-struct sync-wait slots are limited.  Move
    excess waits onto no-op instructions inserted just before, on the same
    engine, preserving execution order semantics."""
    import concourse.mybir as mybir

    nid = [0]
    for f in nc.m.functions:
        for bb in f.blocks:
            out = []
            for inst in bb.instructions:
                si = inst.sync_info
                waits = list(si.on_wait) if si is not None and si.on_wait else []
                cap = (
                    10**9
                    if inst.opcode in _WAIT_SKIP
                    else _WAIT_CAPS.get(inst.opcode, _WAIT_DEFAULT_CAP)
                )
                if len(waits) > cap:
                    excess = waits[: len(waits) - cap]
                    keep = waits[len(waits) - cap :]
                    for j in range(0, len(excess), _NOP_CAP):
                        nop = mybir.InstNoOp(
                            name=f"I-waitsplit-{nid[0]}", ins=[], outs=[]
                        )
                        nid[0] += 1
                        nop.engine = inst.engine
                        nop.bass_nofuse = True
                        nop.sync_info = mybir.SyncInfo(
                            on_wait=excess[j : j + _NOP_CAP], on_update=[]
                        )
                        out.append(nop)
                    inst.sync_info = mybir.SyncInfo(
                        on_wait=keep, on_update=list(si.on_update or [])
                    )
                out.append(inst)
            bb.instructions[:] = out


def build_program(reps=1, hw_loop=0):
    """reps: python-unrolled repetitions.  hw_loop: if >0, wrap the body in a
    Tile For_i hardware loop of that many iterations (for timing runs)."""
    import contextlib
    import concourse.bass as bass
    import concourse.mybir as mybir
    import concourse.tile as tile

    exp2op = _get_exp2_op()

    F32 = mybir.dt.float32
    F16 = mybir.dt.float16
    BF16 = mybir.dt.bfloat16
    I32 = mybir.dt.int32
    AF = mybir.ActivationFunctionType
    ALU = mybir.AluOpType

    nc = bass.Bass(
        "TRN2",
        target_bir_lowering=False,
        debug=False,
        enable_asserts=True,
        num_devices=N_CORES,
    )

    xt_d = nc.dram_tensor("xt", [B, C, N], BF16, kind="ExternalInput").ap()
    wq_d = nc.dram_tensor("wq", [128, 1024], BF16, kind="ExternalInput").ap()
    wk_d = nc.dram_tensor("wk", [128, 1024], BF16, kind="ExternalInput").ap()
    wv_d = nc.dram_tensor("wv", [128, 1024], BF16, kind="ExternalInput").ap()
    pw_d = nc.dram_tensor("pw", [DD, C], BF16, kind="ExternalInput").ap()
    qw_d = nc.dram_tensor("qw", [128, 1], F32, kind="ExternalInput").ap()
    kw_d = nc.dram_tensor("kw", [128, 1], F32, kind="ExternalInput").ap()
    onesrep_d = nc.dram_tensor("onesrep", [128, 128], BF16, kind="ExternalInput").ap()
    y_d = nc.dram_tensor("y", [B, N, C], BF16, kind="ExternalOutput").ap()

    NKC = C // 128  # 8 contraction chunks
    NJC = N // 128  # 16 key chunks
    NIH = 4  # i-quarters of 512
    IW = N // NIH  # 512

    with tile.TileContext(nc) as tc:
        with (
            tc.tile_pool(name="const", bufs=1) as cpool,
            tc.tile_pool(name="xt", bufs=2) as xpool,
            tc.tile_pool(name="qk", bufs=2) as qkpool,
            tc.tile_pool(name="v", bufs=2) as vpool,
            tc.tile_pool(name="work", bufs=2) as wpool,
            tc.tile_pool(name="y", bufs=3) as ypool,
            tc.tile_pool(name="ps", bufs=1, space="PSUM") as ps,
        ):
            # --- constants ---
            w_sb = {}
            for name, dram in (("wq", wq_d), ("wk", wk_d), ("wv", wv_d)):
                t = cpool.tile([128, 1024], BF16, tag=f"c_{name}")
                nc.sync.dma_start(out=t[:], in_=dram)
                w_sb[name] = t
            pw_sb = cpool.tile([DD, C], BF16, tag="c_pw")
            nc.sync.dma_start(out=pw_sb[:], in_=pw_d)
            qw_sb = cpool.tile([128, 1], F32, tag="c_qw")
            nc.sync.dma_start(out=qw_sb[:], in_=qw_d)
            kw_sb = cpool.tile([128, 1], F32, tag="c_kw")
            nc.sync.dma_start(out=kw_sb[:], in_=kw_d)
            onesrep_sb = cpool.tile([128, 128], BF16, tag="c_onesrep")
            nc.sync.dma_start(out=onesrep_sb[:], in_=onesrep_d)
            eps_sb = cpool.tile([128, 1], F32, tag="c_eps")
            nc.vector.memset(eps_sb[:], EPS)
            lnlam_sb = cpool.tile([128, 1], F32, tag="c_lnlam")
            nc.vector.memset(lnlam_sb[:], float(np.log(LAMBDA)))
            expbias_sb = cpool.tile([128, 1], F32, tag="c_expbias")
            nc.vector.memset(expbias_sb[:], ACT_EXP_BIAS)

            # persistent per-batch state handed from gen_qv to gen_a
            state = {}

            def gen_qv(b):
                """QKV projection + RMSNorm + V phase for batch b (PE-heavy)."""
                xt_sb = []
                for kc in range(NKC):
                    t = xpool.tile([128, N], BF16, tag=f"xt{kc}")
                    nc.sync.dma_start(
                        out=t[:], in_=xt_d[b, kc * 128 : (kc + 1) * 128, :]
                    )
                    xt_sb.append(t)
                    yield

                qnT = qkpool.tile([128, N], BF16, tag="qnT")
                knT = qkpool.tile([128, N], BF16, tag="knT")
                qraw = qkpool.tile([128, N], BF16, tag="qraw", bufs=1)
                kraw = qkpool.tile([128, N], BF16, tag="kraw", bufs=1)
                varb = qkpool.tile([128, 2 * N], F32, tag="varb", bufs=1)
                for ti, (wkey, rawT) in enumerate((("wq", qraw), ("wk", kraw))):
                    for ncq in range(N // 512):
                        sl = slice(ncq * 512, (ncq + 1) * 512)
                        vsl = slice(ti * N + ncq * 512, ti * N + (ncq + 1) * 512)
                        pq = ps.tile([128, 512], F32, tag="small0", name="pq")
                        for kc in range(NKC):
                            nc.tensor.matmul(
                                pq[:],
                                w_sb[wkey][:, kc * 128 : (kc + 1) * 128],
                                xt_sb[kc][:, sl],
                                start=(kc == 0),
                                stop=(kc == NKC - 1),
                            )
                        nc.vector.tensor_copy(rawT[:, sl], pq[:])
                        sq = wpool.tile([128, 512], BF16, tag="sq", bufs=4)
                        nc.vector.tensor_mul(sq[:], pq[:], rawT[:, sl])
                        psums = ps.tile([128, 512], F32, tag="small0", name="psums")
                        nc.tensor.matmul(
                            psums[:], onesrep_sb[:], sq[:], start=True, stop=True
                        )
                        nc.vector.tensor_copy(varb[:, vsl], psums[:])
                        yield
                # rsqrt via ln+exp: stays in the natural_log_exp ACT table set.
                # varb = sum(q^2) over head dims; l = ln(varb/D + eps);
                # rvar = exp(-l/2 [+ ln(lambda) for q]) = lambda * var^-1/2
                nc.scalar.activation(
                    varb[:], varb[:], AF.Ln, bias=eps_sb[:], scale=1.0 / D
                )
                yield
                nc.scalar.activation(
                    varb[:, 0:N], varb[:, 0:N], AF.Exp,
                    bias=lnlam_sb[:], scale=-0.5,
                )
                nc.scalar.activation(
                    varb[:, N : 2 * N], varb[:, N : 2 * N], AF.Exp,
                    bias=0.0, scale=-0.5,
                )
                yield
                for ti, (rawT, wcol, dstT) in enumerate(
                    ((qraw, qw_sb, qnT), (kraw, kw_sb, knT))
                ):
                    for ncq in range(N // 512):
                        sl = slice(ncq * 512, (ncq + 1) * 512)
                        vsl = slice(ti * N + ncq * 512, ti * N + (ncq + 1) * 512)
                        nc.vector.scalar_tensor_tensor(
                            dstT[:, sl],
                            rawT[:, sl],
                            wcol[:],
                            varb[:, vsl],
                            op0=ALU.mult,
                            op1=ALU.mult,
                        )
                        yield

                # --- V phase: v in natural layout + ones columns ---
                v_tiles = []
                for jc in range(NJC):
                    pvt = ps.tile([128, 512], F32, tag="small0", name="pv")
                    pv = pvt[:, 0:128]
                    for kc in range(NKC):
                        nc.tensor.matmul(
                            pv,
                            xt_sb[kc][:, jc * 128 : (jc + 1) * 128],
                            w_sb["wv"][:, kc * 128 : (kc + 1) * 128],
                            start=(kc == 0),
                            stop=(kc == NKC - 1),
                        )
                    vt = vpool.tile([128, 130], BF16, tag=f"v{jc}")
                    # one strided copy: cols [0:64] and [65:129]
                    nc.vector.tensor_copy(
                        _strided2(vt[:], 64, 65, start=0), _strided2(pvt[:], 64, 64)
                    )
                    nc.vector.memset(_strided2(vt[:], 1, 65, start=64), 1.0)
                    v_tiles.append(vt)
                    yield
                state[b] = (qnT, knT, v_tiles)

            def gen_a(b):
                """Attention + projection for batch b (ACT/DVE-heavy)."""
                qnT, knT, v_tiles = state.pop(b)
                for ih in range(NIH):
                    isl = slice(ih * IW, (ih + 1) * IW)
                    acc_h = []
                    for h in range(HPC):
                        acc_t = ps.tile([65, IW], F32, tag=f"acc{h}", name=f"acc_t{h}")
                        acc_h.append(acc_t)
                    sc_live = {}

                    def do_scores(jc):
                        scs = ps.tile(
                            [128, 2 * IW], F32, tag="scs", bufs=2, name="scs"
                        )
                        for h in range(HPC):
                            hs = slice(h * 64, (h + 1) * 64)
                            nc.tensor.matmul(
                                scs[:, h * IW : (h + 1) * IW],
                                knT[hs, jc * 128 : (jc + 1) * 128],
                                qnT[hs, isl],
                                start=True,
                                stop=True,
                                tile_position=(h * 64, 0),
                            )
                        sc_live[jc] = scs

                    def do_exp(jc):
                        scs = sc_live.pop(jc)
                        p = wpool.tile([128, 2 * IW], BF16, tag="p", bufs=3)
                        if jc in DVE_JCS[ih]:
                            # DVE path: int32 Schraudolph seed + cubic fixup
                            y0 = wpool.tile([128, 2 * IW], I32, tag="y0", bufs=2)
                            nc.vector.tensor_scalar(
                                y0[:], scs[:], float(2.0**23),
                                float(127 * 2.0**23), ALU.mult, ALU.add,
                            )
                            nc.vector._custom_dve(
                                exp2op,
                                out=p[:],
                                in0=y0[:].bitcast(F32),
                                s0=MANT_MASK,
                                s1=-EXP_A,
                                imm2=EXP_C,
                            )
                        else:
                            nc.scalar.activation(
                                p[:], scs[:], AF.Exp, bias=expbias_sb[:], scale=LN2
                            )
                        return p

                    # scores run two jc ahead of exp so ACT never starves
                    do_scores(0)
                    do_scores(1)
                    p_prev = None
                    for jc in range(NJC):
                        if p_prev is not None:
                            vt = v_tiles[jc - 1]
                            for h in range(HPC):
                                nc.tensor.matmul(
                                    acc_h[h][:],
                                    vt[:, h * 65 : h * 65 + 65],
                                    p_prev[:, h * IW : (h + 1) * IW],
                                    start=(jc == 1),
                                    stop=False,
                                )
                        if jc + 2 < NJC:
                            do_scores(jc + 2)
                        p_prev = do_exp(jc)
                        yield
                    vt = v_tiles[NJC - 1]
                    for h in range(HPC):
                        nc.tensor.matmul(
                            acc_h[h][:],
                            vt[:, h * 65 : h * 65 + 65],
                            p_prev[:, h * IW : (h + 1) * IW],
                            start=False,
                            stop=True,
                        )
                    # drain: normalize + project + store
                    rb2 = wpool.tile([128, IW], F32, tag="rb2", bufs=2)
                    for h in range(HPC):
                        rec = wpool.tile([1, IW], F32, tag=f"rec{h}", bufs=3)
                        nc.vector.reciprocal(rec[:], acc_h[h][64:65, :])
                        nc.sync.dma_start(
                            out=rb2[h * 64 : (h + 1) * 64, :],
                            in_=_row_bcast(rec[0:1, :], 64),
                        )
                    yield
                    outTn = wpool.tile([128, IW], BF16, tag="outTn", bufs=3)
                    for h in range(HPC):
                        nc.vector.tensor_mul(
                            outTn[h * 64 : (h + 1) * 64, :],
                            acc_h[h][0:64, :],
                            rb2[h * 64 : (h + 1) * 64, :],
                        )
                    yield
                    for mc in range(IW // 128):
                        for oc in range(C // 512):
                            yp = ps.tile([128, 512], F32, tag="small1", name="yp")
                            nc.tensor.matmul(
                                yp[:],
                                outTn[:, mc * 128 : (mc + 1) * 128],
                                pw_sb[:, oc * 512 : (oc + 1) * 512],
                                start=True,
                                stop=True,
                            )
                            ysb = ypool.tile([128, 512], BF16, tag="ysb")
                            nc.vector.tensor_copy(ysb[:], yp[:])
                            qi0 = ih * IW + mc * 128
                            nc.sync.dma_start(
                                out=y_d[b, qi0 : qi0 + 128, oc * 512 : (oc + 1) * 512],
                                in_=ysb[:],
                            )
                        yield

            def drive(gen):
                for _ in gen:
                    pass

            def interleave(ga, gq, ratio=2):
                """Step ga `ratio` times per gq step until both exhaust."""
                da = dq = False
                while not (da and dq):
                    for _ in range(ratio):
                        if not da:
                            da = next(ga, _DONE) is _DONE
                    if not dq:
                        dq = next(gq, _DONE) is _DONE

            _DONE = object()

            # software-pipeline rotated across reps/loop-iterations: the
            # prologue computes QV(3); each body iteration runs
            # A(3)xQV(0), A(0)xQV(1), A(1)xQV(2), A(2)xQV(3) so the last
            # batch's attention overlaps the next iteration's QKV phase.
            drive(gen_qv(3))
            loop_ctx = tc.For_i(0, hw_loop, 1) if hw_loop else contextlib.nullcontext()
            with loop_ctx:
                for rep in range(reps):
                    for b in (3, 0, 1, 2):
                        interleave(gen_a(b), gen_qv((b + 1) % B), ratio=2)
    _split_waits(nc)
    return nc


def _prepare_inputs(x, qkv_w, q_norm_w, k_norm_w, proj_w):
    """Host-side sharding/layout prep. Returns per-core input maps."""
    xt = np.ascontiguousarray(x.transpose(0, 2, 1)).astype(bf16)  # [B, C, N]
    qw_col = np.tile(q_norm_w, HPC).reshape(128, 1).astype(np.float32)
    kw_col = np.tile(k_norm_w, HPC).reshape(128, 1).astype(np.float32)
    onesrep = np.zeros((128, 128), bf16)
    onesrep[0:64, 0:64] = 1
    onesrep[64:128, 64:128] = 1

    in_maps = []
    for c in range(N_CORES):
        rows = slice(DD * c, DD * (c + 1))

        def pack(w):  # [128 rows, C] -> packed lhsT chunks [128, 1024]
            chunks = [
                np.ascontiguousarray(w[:, kc * 128 : (kc + 1) * 128].T)
                for kc in range(C // 128)
            ]
            return np.concatenate(chunks, axis=1).astype(bf16)

        in_maps.append(
            {
                "xt": xt,
                "wq": pack(qkv_w[0 * C :][rows, :]),
                "wk": pack(qkv_w[1 * C + DD * c : 1 * C + DD * (c + 1), :]),
                "wv": pack(qkv_w[2 * C + DD * c : 2 * C + DD * (c + 1), :]),
                "pw": np.ascontiguousarray(proj_w[:, rows].T).astype(bf16),
                "qw": qw_col,
                "kw": kw_col,
                "onesrep": onesrep,
            }
        )
    return in_maps


def run_on_device(in_maps, reps=1, hw_loop=0):
    from concourse.bass_utils import run_bass_kernel_spmd

    key = (reps, hw_loop)
    if key not in _COMPILED:
        _COMPILED[key] = build_program(reps, hw_loop=hw_loop)
    nc = _COMPILED[key]
    res = run_bass_kernel_spmd(nc, in_maps, list(range(N_CORES)))
    return res


def kernel(x, qkv_w, q_norm_w, k_norm_w, proj_w, proj_b):
    x = np.asarray(x, np.float32)
    qkv_w = np.asarray(qkv_w, np.float32)
    proj_w = np.asarray(proj_w, np.float32)
    in_maps = _prepare_inputs(
        x, qkv_w, np.asarray(q_norm_w, np.float32), np.asarray(k_norm_w, np.float32), proj_w
    )
    res = run_on_device(in_maps, reps=1)
    y = np.zeros((B, N, C), np.float32)
    for c in range(N_CORES):
        y += np.asarray(res.results[c]["y"], np.float32)
    y += np.asarray(proj_b, np.float32)[None, None, :]
    return y
